# revision 2
# baseline (speedup 1.0000x reference)
"""CrossAttention TRN2 kernel: b=8 sharded across 8 NeuronCores (data parallel).

Per core (b=1): x[1024,1024], y[1024,768] -> out[1024,1024].
  q = x@WqT + bq (softmax scale 1/8 folded into WqT/bq on host)
  kv = y@WkvT + bkv ; per head h: k = rows h*128..+64, v = rows h*128+64..+128
  s^T[m,l] = k^T.T @ q^T ; p = exp(s) (no max subtraction; logits ~N(0,1))
  attn@v via lhsT=[v|ones]: psum rows 0:64 = o^T, rows 64:128 = softmax sums
  o^T head h -> partitions (h%2)*64 of oT tile h//2 after mul by 1/sums
  out = o^T.T @ WoT + bo
All matmuls in float32r (1 cyc/row); biases added via rank-1 (K=1) matmuls.
"""
import numpy as np

import concourse.bass as bass
import concourse.tile as tile
import concourse.mybir as mybir
from concourse import bacc
from concourse.masks import make_identity
from concourse.bass_utils import run_bass_kernel_spmd
from contextlib import ExitStack

FP32 = mybir.dt.float32
FP32R = mybir.dt.float32r
AF = mybir.ActivationFunctionType

B, L, M, D, DC, H = 8, 1024, 1024, 1024, 768, 16


def _body(nc, tc, X, Y, WQT, WKVT, WOT, BQ, BKV, BO, OUT):
    with ExitStack() as ctx:
        setup = ctx.enter_context(tc.tile_pool(name="setup", bufs=1))
        yT_pool = ctx.enter_context(tc.tile_pool(name="yTp", bufs=1))
        qT_pool = ctx.enter_context(tc.tile_pool(name="qTp", bufs=1))
        oT_pool = ctx.enter_context(tc.tile_pool(name="oTp", bufs=1))

        ident = setup.tile([128, 128], FP32, tag="ident")
        make_identity(nc, ident[:])
        ones_f = setup.tile([1, 512], FP32, tag="ones_f")
        nc.gpsimd.memset(ones_f[:], 1.0)
        ones = setup.tile([1, 512], FP32R, tag="ones")
        nc.vector.tensor_copy(ones[:], ones_f[:])
        bq_r = setup.tile([1, D], FP32R, tag="bq")
        nc.sync.dma_start(bq_r[:], BQ[:])
        bkv_r = setup.tile([1, 2 * D], FP32R, tag="bkv")
        nc.sync.dma_start(bkv_r[:], BKV[:])
        bo_r = setup.tile([1, D], FP32R, tag="bo")
        nc.sync.dma_start(bo_r[:], BO[:])

        qT = [qT_pool.tile([128, L], FP32R, tag=f"qT{j}", name=f"qT{j}") for j in range(8)]
        yT = [yT_pool.tile([128, M], FP32R, tag=f"yT{j}", name=f"yT{j}") for j in range(6)]
        oT = [oT_pool.tile([128, L], FP32R, tag=f"oT{j}", name=f"oT{j}") for j in range(8)]

        # ---- Phase A: x -> xT (PE transpose), qT = WqT.T @ xT + bq ----
        with ExitStack() as actx:
            xpool = actx.enter_context(tc.tile_pool(name="xp", bufs=8))
            xT_pool = actx.enter_context(tc.tile_pool(name="xTp", bufs=1))
            wq_pool = actx.enter_context(tc.tile_pool(name="wqp", bufs=2))
            ps_t = actx.enter_context(
                tc.tile_pool(name="ps_t", bufs=2, space="PSUM"))
            ps_q = actx.enter_context(
                tc.tile_pool(name="ps_q", bufs=2, space="PSUM"))

            xT = [xT_pool.tile([128, L], FP32R, tag=f"xT{j}", name=f"xT{j}") for j in range(8)]
            x_tiles = []
            for i in range(8):
                xt = xpool.tile([128, D], FP32, tag="x")
                nc.sync.dma_start(xt[:], X[i * 128:(i + 1) * 128, :])
                x_tiles.append(xt)
            for j in range(8):
                for i4 in range(2):
                    pt_ = ps_t.tile([128, 512], FP32, tag="pst")
                    for i in range(4):
                        nc.tensor.transpose(
                            pt_[:, i * 128:(i + 1) * 128],
                            x_tiles[i4 * 4 + i][:, j * 128:(j + 1) * 128],
                            ident[:])
                    nc.vector.tensor_copy(
                        xT[j][:, i4 * 512:(i4 + 1) * 512], pt_[:])

            WQT_r = WQT[:].rearrange("(ko p) e -> p ko e", p=128)
            for et in range(8):
                wq = wq_pool.tile([128, 8, 128], FP32R, tag="wq")
                nc.sync.dma_start(wq[:], WQT_r[:, :, et * 128:(et + 1) * 128])
                for lh in range(2):
                    pq = ps_q.tile([128, 512], FP32, tag="psq")
                    for k in range(8):
                        nc.tensor.matmul(
                            pq[:], wq[:, k, :],
                            xT[k][:, lh * 512:(lh + 1) * 512],
                            start=(k == 0), stop=False)
                    nc.tensor.matmul(
                        pq[:], bq_r[:, et * 128:(et + 1) * 128], ones[:],
                        start=False, stop=True)
                    nc.scalar.activation(
                        qT[et][:, lh * 512:(lh + 1) * 512], pq[:], AF.Copy)

            # ---- y -> yT ----
            y_tiles = []
            for i in range(8):
                yt = xpool.tile([128, DC], FP32, tag="y")
                nc.sync.dma_start(yt[:], Y[i * 128:(i + 1) * 128, :])
                y_tiles.append(yt)
            for j in range(6):
                for i4 in range(2):
                    pt_ = ps_t.tile([128, 512], FP32, tag="pst")
                    for i in range(4):
                        nc.tensor.transpose(
                            pt_[:, i * 128:(i + 1) * 128],
                            y_tiles[i4 * 4 + i][:, j * 128:(j + 1) * 128],
                            ident[:])
                    nc.vector.tensor_copy(
                        yT[j][:, i4 * 512:(i4 + 1) * 512], pt_[:])

        # ---- Phase B: per head: kv proj, vones, attention, normalize ----
        with ExitStack() as bctx:
            kt_pool = bctx.enter_context(tc.tile_pool(name="ktp", bufs=2))
            vto_pool = bctx.enter_context(tc.tile_pool(name="vtop", bufs=2))
            von_pool = bctx.enter_context(tc.tile_pool(name="vonp", bufs=2))
            wkv_pool = bctx.enter_context(tc.tile_pool(name="wkvp", bufs=2))
            pt_pool = bctx.enter_context(tc.tile_pool(name="ptp", bufs=4))
            nrm_pool = bctx.enter_context(tc.tile_pool(name="nrmp", bufs=2))
            ps_big = bctx.enter_context(
                tc.tile_pool(name="ps_big", bufs=3, space="PSUM"))
            ps_kv = bctx.enter_context(
                tc.tile_pool(name="ps_kv", bufs=2, space="PSUM"))

            WKVT_r = WKVT[:].rearrange("(ko p) e -> p ko e", p=128)
            for hp in range(8):
                kt = kt_pool.tile([128, M], FP32R, tag="kt")
                for sub in range(2):
                    h = hp * 2 + sub
                    wkv = wkv_pool.tile([128, 6, 128], FP32R, tag="wkv")
                    nc.sync.dma_start(
                        wkv[:], WKVT_r[:, :, h * 128:(h + 1) * 128])
                    vto = vto_pool.tile([128, M], FP32, tag="vto")
                    nc.gpsimd.memset(vto[64:128, :], 1.0)
                    for mh in range(2):
                        pkv = ps_kv.tile([128, 512], FP32, tag="pkv")
                        for k in range(6):
                            nc.tensor.matmul(
                                pkv[:], wkv[:, k, :],
                                yT[k][:, mh * 512:(mh + 1) * 512],
                                start=(k == 0), stop=False)
                        nc.tensor.matmul(
                            pkv[:], bkv_r[:, h * 128:(h + 1) * 128], ones[:],
                            start=False, stop=True)
                        nc.vector.tensor_copy(
                            kt[sub * 64:sub * 64 + 64,
                               mh * 512:(mh + 1) * 512],
                            pkv[0:64, :])
                        nc.vector.tensor_copy(
                            vto[0:64, mh * 512:(mh + 1) * 512],
                            pkv[64:128, :])
                    vones = von_pool.tile([128, M], FP32R, tag="vones")
                    for j2 in range(2):
                        pvt = ps_kv.tile([128, 512], FP32, tag="pkv")
                        for j in range(4):
                            jj = j2 * 4 + j
                            nc.tensor.transpose(
                                pvt[:, j * 128:(j + 1) * 128],
                                vto[:, jj * 128:(jj + 1) * 128], ident[:])
                        nc.vector.tensor_copy(
                            vones[:, j2 * 512:(j2 + 1) * 512], pvt[:])

                    # attention for head h
                    po = ps_big.tile([128, 1024], FP32, tag="big")
                    for mc in range(8):
                        pss = ps_big.tile([128, 1024], FP32, tag="big")
                        for lh in range(2):
                            nc.tensor.matmul(
                                pss[:, lh * 512:(lh + 1) * 512],
                                kt[sub * 64:sub * 64 + 64,
                                   mc * 128:(mc + 1) * 128],
                                qT[hp][sub * 64:sub * 64 + 64,
                                       lh * 512:(lh + 1) * 512],
                                start=True, stop=True)
                        ptile = pt_pool.tile([128, 1024], FP32R, tag="pt")
                        nc.scalar.activation(ptile[:], pss[:], AF.Exp)
                        for lh in range(2):
                            nc.tensor.matmul(
                                po[:, lh * 512:(lh + 1) * 512],
                                vones[:, mc * 128:(mc + 1) * 128],
                                ptile[:, lh * 512:(lh + 1) * 512],
                                start=(mc == 0), stop=(mc == 7))
                    # normalize: rows 64:128 of po hold the softmax sums
                    stage = nrm_pool.tile([128, 1024], FP32, tag="stage")
                    nc.vector.tensor_copy(stage[64:128, :], po[0:64, :])
                    lns = nrm_pool.tile([128, 1024], FP32, tag="lns")
                    nc.scalar.activation(lns[64:128, :], po[64:128, :], AF.Ln)
                    rec = nrm_pool.tile([128, 1024], FP32, tag="rec")
                    nc.scalar.activation(
                        rec[64:128, :], lns[64:128, :], AF.Exp, scale=-1.0)
                    nc.vector.tensor_mul(
                        oT[hp][sub * 64:sub * 64 + 64, :],
                        stage[64:128, :], rec[64:128, :])

        # ---- Phase C: out = oT.T @ WoT + bo ----
        with ExitStack() as cctx:
            wo_pool = cctx.enter_context(tc.tile_pool(name="wop", bufs=1))
            os_pool = cctx.enter_context(tc.tile_pool(name="osp", bufs=2))
            ps_o = cctx.enter_context(
                tc.tile_pool(name="ps_o", bufs=4, space="PSUM"))
            wo = [wo_pool.tile([128, D], FP32R, tag=f"wo{k}", name=f"wo{k}") for k in range(8)]
            for k in range(8):
                nc.sync.dma_start(wo[k][:], WOT[k * 128:(k + 1) * 128, :])
            for lt in range(8):
                osb = os_pool.tile([128, D], FP32, tag="osb")
                for eh in range(2):
                    po2 = ps_o.tile([128, 512], FP32, tag="pso")
                    for k in range(8):
                        nc.tensor.matmul(
                            po2[:], oT[k][:, lt * 128:(lt + 1) * 128],
                            wo[k][:, eh * 512:(eh + 1) * 512],
                            start=(k == 0), stop=False)
                    nc.tensor.matmul(
                        po2[:], ones[:, 0:128],
                        bo_r[:, eh * 512:(eh + 1) * 512],
                        start=False, stop=True)
                    nc.scalar.activation(
                        osb[:, eh * 512:(eh + 1) * 512], po2[:], AF.Copy)
                nc.sync.dma_start(OUT[lt * 128:(lt + 1) * 128, :], osb[:])


_NC = None


def _build():
    global _NC
    if _NC is not None:
        return _NC
    nc = bacc.Bacc("TRN2", target_bir_lowering=False, debug=False,
                   num_devices=8)
    X = nc.dram_tensor("x", [L, D], FP32, kind="ExternalInput")
    Y = nc.dram_tensor("y", [M, DC], FP32, kind="ExternalInput")
    WQT = nc.dram_tensor("wqt", [D, D], FP32R, kind="ExternalInput")
    WKVT = nc.dram_tensor("wkvt", [DC, 2 * D], FP32R, kind="ExternalInput")
    WOT = nc.dram_tensor("wot", [D, D], FP32R, kind="ExternalInput")
    BQ = nc.dram_tensor("bq", [1, D], FP32R, kind="ExternalInput")
    BKV = nc.dram_tensor("bkv", [1, 2 * D], FP32R, kind="ExternalInput")
    BO = nc.dram_tensor("bo", [1, D], FP32R, kind="ExternalInput")
    OUT = nc.dram_tensor("out", [L, D], FP32, kind="ExternalOutput")
    with tile.TileContext(nc) as tc:
        _body(nc, tc, X, Y, WQT, WKVT, WOT, BQ, BKV, BO, OUT)
    nc.compile()
    _NC = nc
    return nc


def _in_maps(x, y, Wq, bq, Wkv, bkv, Wo, bo):
    x = np.asarray(x, np.float32)
    y = np.asarray(y, np.float32)
    wqt = np.ascontiguousarray(np.asarray(Wq, np.float32).T / 8.0)
    bqs = (np.asarray(bq, np.float32) / 8.0).reshape(1, D)
    wkvt = np.ascontiguousarray(np.asarray(Wkv, np.float32).T)
    bkvr = np.asarray(bkv, np.float32).reshape(1, 2 * D)
    wot = np.ascontiguousarray(np.asarray(Wo, np.float32).T)
    bor = np.asarray(bo, np.float32).reshape(1, D)
    return [
        dict(x=np.ascontiguousarray(x[i]), y=np.ascontiguousarray(y[i]),
             wqt=wqt, wkvt=wkvt, wot=wot, bq=bqs, bkv=bkvr, bo=bor)
        for i in range(B)
    ]


def kernel_run(trace=False, **inputs):
    nc = _build()
    res = run_bass_kernel_spmd(
        nc, _in_maps(**inputs), list(range(B)), trace=trace)
    out = np.stack([res.results[i]["out"] for i in range(B)])
    return out.astype(np.float32), res


def kernel(**inputs):
    out, _ = kernel_run(trace=False, **inputs)
    return out


# revision 11
# speedup vs baseline: 1.0608x; 1.0608x over previous
"""CrossAttention TRN2 kernel: b=8 sharded across 8 NeuronCores (data parallel).

Per core (b=1): x[1024,1024], y[1024,768] -> out[1024,1024].
  q = x@WqT + bq (softmax scale 1/8 folded into WqT/bq on host)
  kv = y@WkvT + bkv ; per head h: k = rows h*128..+64, v = rows h*128+64..+128
  s^T[m,l] = k^T.T @ q^T ; p = exp(s) (no max subtraction; logits ~N(0,1))
  attn@v via lhsT=[v|ones]: psum rows 0:64 = o^T, rows 64:128 = softmax sums
  o^T head h -> partitions (h%2)*64 of oT tile h//2 after mul by 1/sums
  out = o^T.T @ WoT + bo
All matmuls in float32r (1 cyc/row); biases added via rank-1 (K=1) matmuls.
"""
import os
import numpy as np

import concourse.bass as bass
import concourse.tile as tile
import concourse.mybir as mybir
from concourse import bacc
from concourse.masks import make_identity
from concourse.bass_utils import run_bass_kernel_spmd
from contextlib import ExitStack

FP32 = mybir.dt.float32
FP32R = mybir.dt.float32r
AF = mybir.ActivationFunctionType

B, L, M, D, DC, H = 8, 1024, 1024, 1024, 768, 16
_SKIP_LOADS = bool(os.environ.get("KERNEL_SKIP_LOADS"))


def _load(nc, dst, src_ap):
    if not _SKIP_LOADS:
        nc.sync.dma_start(dst, src_ap)


def _normalize(nc, nrm_pool, po, oT_tile, sub):
    """v1-proven chain, all DVE/ACT partition offsets probe-validated:
    stage[64:] <- po[0:64] (DVE cross); Ln/Exp aligned on ACT; mul on DVE."""
    stage = nrm_pool.tile([128, 1024], FP32, tag="stage")
    nc.vector.tensor_copy(stage[64:128, :], po[0:64, :])
    lns = nrm_pool.tile([128, 1024], FP32, tag="lns")
    nc.scalar.activation(lns[64:128, :], po[64:128, :], AF.Ln)
    rec = nrm_pool.tile([128, 1024], FP32, tag="rec")
    nc.scalar.activation(rec[64:128, :], lns[64:128, :], AF.Exp, scale=-1.0)
    nc.vector.tensor_mul(
        oT_tile[sub * 64:sub * 64 + 64, :],
        stage[64:128, :], rec[64:128, :])


def _body(nc, tc, X, Y, WQT, WKVT, WOT, BQ, BKV, BO, OUT):
    with ExitStack() as ctx:
        setup = ctx.enter_context(tc.tile_pool(name="setup", bufs=1))
        yT_pool = ctx.enter_context(tc.tile_pool(name="yTp", bufs=1))
        qT_pool = ctx.enter_context(tc.tile_pool(name="qTp", bufs=1))
        oT_pool = ctx.enter_context(tc.tile_pool(name="oTp", bufs=1))

        ident = setup.tile([128, 128], FP32, tag="ident")
        make_identity(nc, ident[:])
        ones_f = setup.tile([1, 512], FP32, tag="ones_f")
        nc.gpsimd.memset(ones_f[:], 1.0)
        ones = setup.tile([1, 512], FP32R, tag="ones")
        nc.vector.tensor_copy(ones[:], ones_f[:])
        bq_r = setup.tile([128, 8], FP32, tag="bq")
        nc.sync.dma_start(bq_r[:], BQ[:])
        bkv_r = setup.tile([128, 16], FP32, tag="bkv")
        nc.sync.dma_start(bkv_r[:], BKV[:])
        bo_r = setup.tile([1, D], FP32R, tag="bo")
        nc.sync.dma_start(bo_r[:], BO[:])

        qT = [qT_pool.tile([128, L], FP32R, tag=f"qT{j}", name=f"qT{j}") for j in range(8)]
        yT = [yT_pool.tile([128, M], FP32R, tag=f"yT{j}", name=f"yT{j}") for j in range(6)]
        oT = [oT_pool.tile([128, L], FP32R, tag=f"oT{j}", name=f"oT{j}") for j in range(8)]

        # ---- Phase A: x -> xT (PE transpose), qT = WqT.T @ xT + bq ----
        with ExitStack() as actx:
            xpool = actx.enter_context(tc.tile_pool(name="xp", bufs=8))
            xT_pool = actx.enter_context(tc.tile_pool(name="xTp", bufs=1))
            wq_pool = actx.enter_context(tc.tile_pool(name="wqp", bufs=2))
            ps_t = actx.enter_context(
                tc.tile_pool(name="ps_t", bufs=2, space="PSUM"))
            ps_q = actx.enter_context(
                tc.tile_pool(name="ps_q", bufs=2, space="PSUM"))

            xT = [xT_pool.tile([128, L], FP32R, tag=f"xT{j}", name=f"xT{j}") for j in range(8)]
            x_tiles = []
            for i in range(8):
                xt = xpool.tile([128, D], FP32, tag="x")
                _load(nc, xt[:], X[i * 128:(i + 1) * 128, :])
                x_tiles.append(xt)
            for j in range(8):
                for i4 in range(2):
                    pt_ = ps_t.tile([128, 512], FP32, tag="pst")
                    for i in range(4):
                        nc.tensor.transpose(
                            pt_[:, i * 128:(i + 1) * 128],
                            x_tiles[i4 * 4 + i][:, j * 128:(j + 1) * 128],
                            ident[:])
                    nc.vector.tensor_copy(
                        xT[j][:, i4 * 512:(i4 + 1) * 512], pt_[:])

            WQT_r = WQT[:].rearrange("(ko p) e -> p ko e", p=128)
            for et in range(8):
                wq = wq_pool.tile([128, 8, 128], FP32R, tag="wq")
                _load(nc, wq[:], WQT_r[:, :, et * 128:(et + 1) * 128])
                for lh in range(2):
                    pq = ps_q.tile([128, 512], FP32, tag="psq")
                    for k in range(8):
                        nc.tensor.matmul(
                            pq[:], wq[:, k, :],
                            xT[k][:, lh * 512:(lh + 1) * 512],
                            start=(k == 0), stop=(k == 7))
                    nc.scalar.activation(
                        qT[et][:, lh * 512:(lh + 1) * 512], pq[:],
                        AF.Identity, bias=bq_r[:, et:et + 1])

            # ---- y -> yT ----
            y_tiles = []
            for i in range(8):
                yt = xpool.tile([128, DC], FP32, tag="y")
                _load(nc, yt[:], Y[i * 128:(i + 1) * 128, :])
                y_tiles.append(yt)
            for j in range(6):
                for i4 in range(2):
                    pt_ = ps_t.tile([128, 512], FP32, tag="pst")
                    for i in range(4):
                        nc.tensor.transpose(
                            pt_[:, i * 128:(i + 1) * 128],
                            y_tiles[i4 * 4 + i][:, j * 128:(j + 1) * 128],
                            ident[:])
                    nc.vector.tensor_copy(
                        yT[j][:, i4 * 512:(i4 + 1) * 512], pt_[:])

        # Wo loads hoisted: prefetch during attention (no address overlap
        # with phase-B pools since this pool lives in the outer scope).
        wo_pool = ctx.enter_context(tc.tile_pool(name="wop", bufs=1))
        wo = [wo_pool.tile([128, D], FP32R, tag=f"wo{k}", name=f"wo{k}")
              for k in range(8)]
        for k in range(8):
            _load(nc, wo[k][:], WOT[k * 128:(k + 1) * 128, :])

        # ---- Phase B: per head: kv proj, vones, attention, normalize ----
        with ExitStack() as bctx:
            kt_pool = bctx.enter_context(tc.tile_pool(name="ktp", bufs=2))
            vto_pool = bctx.enter_context(tc.tile_pool(name="vtop", bufs=3))
            von_pool = bctx.enter_context(tc.tile_pool(name="vonp", bufs=3))
            wkv_pool = bctx.enter_context(tc.tile_pool(name="wkvp", bufs=4))
            pt_pool = bctx.enter_context(tc.tile_pool(name="ptp", bufs=5))
            nrm_pool = bctx.enter_context(tc.tile_pool(name="nrmp", bufs=1))
            ps_big = bctx.enter_context(
                tc.tile_pool(name="ps_big", bufs=3, space="PSUM"))
            ps_kv = bctx.enter_context(
                tc.tile_pool(name="ps_kv", bufs=2, space="PSUM"))

            WKVT_r = WKVT[:].rearrange("(ko p) e -> p ko e", p=128)
            pending = None  # (po, hp, sub) normalization deferred one head
            for hp in range(8):
                kt = kt_pool.tile([128, M], FP32R, tag="kt")
                for sub in range(2):
                    h = hp * 2 + sub
                    wkv = wkv_pool.tile([128, 6, 128], FP32R, tag="wkv")
                    _load(nc, wkv[:], WKVT_r[:, :, h * 128:(h + 1) * 128])
                    vto = vto_pool.tile([128, M], FP32, tag="vto")
                    nc.gpsimd.memset(vto[64:128, :], 1.0)
                    for mh in range(2):
                        pkv = ps_kv.tile([128, 512], FP32, tag="pkv")
                        for k in range(6):
                            nc.tensor.matmul(
                                pkv[:], wkv[:, k, :],
                                yT[k][:, mh * 512:(mh + 1) * 512],
                                start=(k == 0), stop=(k == 5))
                        nc.vector.tensor_scalar_add(
                            kt[sub * 64:sub * 64 + 64,
                               mh * 512:(mh + 1) * 512],
                            pkv[0:64, :], bkv_r[0:64, h:h + 1])
                        nc.vector.tensor_scalar_add(
                            vto[0:64, mh * 512:(mh + 1) * 512],
                            pkv[64:128, :], bkv_r[64:128, h:h + 1])
                    vones = von_pool.tile([128, M], FP32R, tag="vones")
                    for j2 in range(2):
                        pvt = ps_kv.tile([128, 512], FP32, tag="pkv")
                        for j in range(4):
                            jj = j2 * 4 + j
                            nc.tensor.transpose(
                                pvt[:, j * 128:(j + 1) * 128],
                                vto[:, jj * 128:(jj + 1) * 128], ident[:])
                        nc.vector.tensor_copy(
                            vones[:, j2 * 512:(j2 + 1) * 512], pvt[:])

                    # normalize the PREVIOUS head here so its DVE ops
                    # queue behind this head's kv/vones copies (which gate PE)
                    if pending is not None:
                        p_po, p_hp, p_sub = pending
                        _normalize(nc, nrm_pool, p_po, oT[p_hp], p_sub)
                        pending = None
                    # attention for head h
                    po = ps_big.tile([128, 1024], FP32, tag="big")
                    prev_pt = None
                    for mc in range(8):
                        pss = ps_big.tile([128, 1024], FP32, tag="big")
                        for lh in range(2):
                            nc.tensor.matmul(
                                pss[:, lh * 512:(lh + 1) * 512],
                                kt[sub * 64:sub * 64 + 64,
                                   mc * 128:(mc + 1) * 128],
                                qT[hp][sub * 64:sub * 64 + 64,
                                       lh * 512:(lh + 1) * 512],
                                start=True, stop=True)
                        ptile = pt_pool.tile([128, 1024], FP32R, tag="pt")
                        nc.scalar.activation(ptile[:], pss[:], AF.Exp)
                        # software pipeline: av for mc-1 issues after sT/exp of
                        # mc so the FIFO PE queue never head-of-line blocks on
                        # the exp the av depends on.
                        if prev_pt is not None:
                            for lh in range(2):
                                nc.tensor.matmul(
                                    po[:, lh * 512:(lh + 1) * 512],
                                    vones[:, (mc - 1) * 128:mc * 128],
                                    prev_pt[:, lh * 512:(lh + 1) * 512],
                                    start=(mc == 1), stop=False)
                        prev_pt = ptile
                    for lh in range(2):
                        nc.tensor.matmul(
                            po[:, lh * 512:(lh + 1) * 512],
                            vones[:, 7 * 128:8 * 128],
                            prev_pt[:, lh * 512:(lh + 1) * 512],
                            start=False, stop=True)
                    pending = (po, hp, sub)
            # flush the last head's normalization
            if pending is not None:
                p_po, p_hp, p_sub = pending
                _normalize(nc, nrm_pool, p_po, oT[p_hp], p_sub)

        # ---- Phase C: out = oT.T @ WoT + bo ----
        with ExitStack() as cctx:
            os_pool = cctx.enter_context(tc.tile_pool(name="osp", bufs=4))
            ps_o = cctx.enter_context(
                tc.tile_pool(name="ps_o", bufs=4, space="PSUM"))
            for lt in range(8):
                osb = os_pool.tile([128, D], FP32, tag="osb")
                for eh in range(2):
                    po2 = ps_o.tile([128, 512], FP32, tag="pso")
                    for k in range(8):
                        nc.tensor.matmul(
                            po2[:], oT[k][:, lt * 128:(lt + 1) * 128],
                            wo[k][:, eh * 512:(eh + 1) * 512],
                            start=(k == 0), stop=False)
                    nc.tensor.matmul(
                        po2[:], ones[:, 0:128],
                        bo_r[:, eh * 512:(eh + 1) * 512],
                        start=False, stop=True)
                    nc.scalar.activation(
                        osb[:, eh * 512:(eh + 1) * 512], po2[:], AF.Copy)
                    nc.sync.dma_start(
                        OUT[lt * 128:(lt + 1) * 128,
                            eh * 512:(eh + 1) * 512],
                        osb[:, eh * 512:(eh + 1) * 512])


_NC = None


def _build():
    global _NC
    if _NC is not None:
        return _NC
    nc = bacc.Bacc("TRN2", target_bir_lowering=False, debug=False,
                   num_devices=8)
    X = nc.dram_tensor("x", [L, D], FP32, kind="ExternalInput")
    Y = nc.dram_tensor("y", [M, DC], FP32, kind="ExternalInput")
    WQT = nc.dram_tensor("wqt", [D, D], FP32R, kind="ExternalInput")
    WKVT = nc.dram_tensor("wkvt", [DC, 2 * D], FP32R, kind="ExternalInput")
    WOT = nc.dram_tensor("wot", [D, D], FP32R, kind="ExternalInput")
    BQ = nc.dram_tensor("bq", [128, 8], FP32, kind="ExternalInput")
    BKV = nc.dram_tensor("bkv", [128, 16], FP32, kind="ExternalInput")
    BO = nc.dram_tensor("bo", [1, D], FP32R, kind="ExternalInput")
    OUT = nc.dram_tensor("out", [L, D], FP32, kind="ExternalOutput")
    with tile.TileContext(nc) as tc:
        _body(nc, tc, X, Y, WQT, WKVT, WOT, BQ, BKV, BO, OUT)
    nc.compile()
    _NC = nc
    return nc


def _in_maps(x, y, Wq, bq, Wkv, bkv, Wo, bo):
    x = np.asarray(x, np.float32)
    y = np.asarray(y, np.float32)
    wqt = np.ascontiguousarray(np.asarray(Wq, np.float32).T / 8.0)
    bqs = np.ascontiguousarray(
        (np.asarray(bq, np.float32) / 8.0).reshape(8, 128).T)
    wkvt = np.ascontiguousarray(np.asarray(Wkv, np.float32).T)
    bkvr = np.ascontiguousarray(
        np.asarray(bkv, np.float32).reshape(16, 128).T)
    wot = np.ascontiguousarray(np.asarray(Wo, np.float32).T)
    bor = np.asarray(bo, np.float32).reshape(1, D)
    return [
        dict(x=np.ascontiguousarray(x[i]), y=np.ascontiguousarray(y[i]),
             wqt=wqt, wkvt=wkvt, wot=wot, bq=bqs, bkv=bkvr, bo=bor)
        for i in range(B)
    ]


def kernel_run(trace=False, **inputs):
    nc = _build()
    res = run_bass_kernel_spmd(
        nc, _in_maps(**inputs), list(range(B)), trace=trace)
    out = np.stack([res.results[i]["out"] for i in range(B)])
    return out.astype(np.float32), res


def kernel(**inputs):
    out, _ = kernel_run(trace=False, **inputs)
    return out
